# revision 43
# baseline (speedup 1.0000x reference)
"""Trainium2 Bass kernel for CompositionalTwoArmedAgent (DND-LSTM A2C step).

Strategy (8 NeuronCores, SPMD, zero collectives):
  - The DND softmax weights w = softmax(cos(keys, q)) depend only on the
    tiny keys table (100000 x 10) and x_t, so the host computes them
    exactly (f32) and uploads the scaled weights in fp8 to every core.
  - vals (100000 x 1024, 400 MB f32) dominates HBM traffic. It is
    sharded COLUMN-wise: core k owns vals[:, 128k:128k+128] in fp8
    (12.85 MB/core) and computes its own 128-dim slice of
    m_t/c_t/h_t with no cross-core reduction at all.
  - The matvec p = w @ vals_slice streams vals through the PE array with
    fp8 DoubleRow matmuls: moving [128, 2, 512] consumes 8 row-chunks
    per instruction (2 k-tiles x 4 block-diagonal chunks packed into the
    512 free columns), 2x the bf16 column rate.  Two PSUM banks
    alternate; the 4 diagonal blocks are extracted at the end with
    [4,128]->[128,4] PE transposes.
  - The LSTM preact is computed per-core only for the 5 gate rows that
    core's slice needs (W rows {s, H+s, 2H+s, 3H+s, 4H+s}), so the
    1.25 MB/core weight load and the gate math also need no collective.
  - A2C head: each core outputs W_ih[:, slice] @ h_t_slice; the host
    sums the 8 partials, applies relu and the tiny (3 x 1024) actor/
    critic matvecs, the 2-class softmax and the fixed-key categorical
    sample (host postprocessing as in the original baseline).
"""

import ml_dtypes
import numpy as np

import concourse.bacc as bacc
import concourse.bass as bass
import concourse.mybir as mybir
import concourse.tile as tile
from concourse.bass_utils import run_bass_kernel_spmd

N_CORES = 8
D, RD, H, IN_DIM, A = 100000, 10, 1024, 14, 2
CH = 784               # 128-row chunks after padding (multiple of 8)
NT = CH // 8           # 98 DoubleRow matmuls (8 chunks each)
PAD_ROWS = CH * 128    # 100352
# groups (8 chunks) per vals DMA block: small first blocks so the first
# matmul fires early (TensorE trails DMA at the end, so big tail blocks
# are free).
BLOCKS = [2, 4, 8, 14, 14, 14, 14, 14, 14]
W5_SCALE = 16.0        # fp8 range scaling for the preact weights
H_SCALE = 8.0          # fp8 range scaling for the h/x stationary vector
F32 = mybir.dt.float32
F32R = mybir.dt.float32r
BF16 = mybir.dt.bfloat16
F16 = mybir.dt.float16
F8 = mybir.dt.float8e4
FP8 = ml_dtypes.float8_e4m3

# jax.random.gumbel(jax.random.key(1), (2,), float32) — fixed constants of the
# reference's categorical sample (verified against jax.random.categorical).
GUMBEL = np.array([0.5325072, -0.01641824], np.float32)

_CACHE = {}


def _input_specs():
    return [
        ("vals_s", [128, CH * 128], F8),   # chunk-tiled fp8 vals column-slice
        ("w_t", [128, NT * 64], F8),       # scaled softmax weights, [t, 2, 32]
                                           # layout (ISA needs >=32 stat cols)
        ("cst16", [128, 8 * 640 + 8], F16),  # [w5t(5120) | h_cols(8)]
        ("cstx", [IN_DIM, 641], F16),        # [wxt(640) | x_col(1)]
        ("cst32", [128, 23], F32),         # [b5t(5)|c2(1)|winv(1)|perm4x4(16)]
        ("wiht", [128, H], F32R),          # W_ih[:, slice].T moving layout
    ]


def _build():
    nc = bacc.Bacc("TRN2", target_bir_lowering=False, debug=False,
                   num_devices=N_CORES)
    d = {name: nc.dram_tensor(name, shp, dt, kind="ExternalInput")
         for name, shp, dt in _input_specs()}
    out_hc = nc.dram_tensor("out_hc", [128, 2], F32, kind="ExternalOutput")
    out_hh = nc.dram_tensor("out_hh", [1, H], F32, kind="ExternalOutput")

    AF = mybir.ActivationFunctionType
    OP = mybir.AluOpType
    DR = mybir.MatmulPerfMode.DoubleRow

    with tile.TileContext(nc) as tc:
        with (
            tc.tile_pool(name="const", bufs=1) as cp,
            tc.tile_pool(name="vals", bufs=3) as vp,
            tc.tile_pool(name="ps", bufs=1, space="PSUM") as pp,
        ):
            # ---- persistent loads --------------------------------------
            # w_t rides the otherwise-idle vector queue so the first
            # matmul's stationary data lands before vals block 0 does
            w_sb = cp.tile([128, NT, 2, 32], F8)
            nc.scalar.dma_start(
                w_sb[:], d["w_t"][:].rearrange("p (t i m) -> p t i m",
                                               i=2, m=32))
            cst32 = cp.tile([128, 23], F32)
            nc.scalar.dma_start(cst32[:], d["cst32"][:])
            cst16 = cp.tile([128, 8 * 640 + 8], F16)
            nc.scalar.dma_start(cst16[:], d["cst16"][:])
            cstx = cp.tile([IN_DIM, 641], F16)
            nc.scalar.dma_start(cstx[:], d["cstx"][:])
            wiht_sb = cp.tile([128, H], F32R)
            nc.scalar.dma_start(wiht_sb[:], d["wiht"][:])

            ones11 = cp.tile([1, 1], F32)
            nc.vector.memset(ones11[:], 1.0)

            # ---- big matvec: p = w @ vals_slice (fp8 DoubleRow) --------
            P0 = pp.tile([32, 512], F32, tag="mv0")
            P1 = pp.tile([32, 512], F32, tag="mv1")
            t = 0
            for bi, nb in enumerate(BLOCKS):
                v = vp.tile([128, nb, 2, 512], F8, tag="v")
                src = d["vals_s"][:, t * 1024:(t + nb) * 1024]
                eng = nc.sync
                eng.dma_start(
                    v[:], src.rearrange("p (g i n) -> p g i n", i=2, n=512))
                for j in range(nb):
                    ps = P0 if (t % 2 == 0) else P1
                    nc.tensor.matmul(ps[:], w_sb[:, t], v[:, j],
                                     start=(t < 2), stop=(t >= NT - 2),
                                     perf_mode=DR)
                    t += 1
                if bi == 2:
                    # preact (f16 moving-operand matmuls) + gate math,
                    # hidden inside the stream while DMA is ahead
                    preA = pp.tile([1, 512], F32, tag="preA")
                    preB = pp.tile([1, 128], F32, tag="preB")
                    for c in range(8):
                        h_col = cst16[:, 8 * 640 + c:8 * 640 + c + 1]
                        nc.tensor.matmul(preA[:], h_col,
                                         cst16[:, c * 640:c * 640 + 512],
                                         start=(c == 0), stop=False)
                        nc.tensor.matmul(preB[:], h_col,
                                         cst16[:, c * 640 + 512:(c + 1) * 640],
                                         start=(c == 0), stop=False)
                    nc.tensor.matmul(preA[:], cstx[:, 640:641],
                                     cstx[:, 0:512], start=False, stop=True)
                    nc.tensor.matmul(preB[:], cstx[:, 640:641],
                                     cstx[:, 512:640], start=False, stop=True)
                    pre_row = cp.tile([1, 640], F32)
                    nc.vector.tensor_copy(pre_row[0:1, 0:512], preA[:])
                    nc.vector.tensor_copy(pre_row[0:1, 512:640], preB[:])
                    psum_preT = pp.tile([128, 5], F32, tag="preT")
                    for n in range(5):
                        nc.tensor.transpose(psum_preT[:, n:n + 1],
                                            pre_row[0:1, n * 128:(n + 1) * 128],
                                            ones11[:])
                    prefull = cp.tile([128, 5], F32)
                    nc.vector.tensor_add(prefull[:], psum_preT[:],
                                         cst32[:, 0:5])
                    th = cp.tile([128, 4], F32)
                    nc.scalar.activation(th[:], prefull[:, 0:4], AF.Tanh,
                                         scale=0.5)
                    gates = cp.tile([128, 4], F32)
                    nc.vector.tensor_scalar(gates[:], th[:], 0.5, 0.5,
                                            OP.mult, OP.add)
                    cnew = cp.tile([128, 1], F32)
                    nc.scalar.activation(cnew[:], prefull[:, 4:5], AF.Tanh)
                    t1 = cp.tile([128, 1], F32)
                    nc.vector.tensor_mul(t1[:], gates[:, 0:1], cst32[:, 5:6])
                    t2 = cp.tile([128, 1], F32)
                    nc.vector.tensor_mul(t2[:], gates[:, 1:2], cnew[:])
                    ct0 = cp.tile([128, 1], F32)
                    nc.vector.tensor_add(ct0[:], t1[:], t2[:])

            # ---- extract the 4 diagonal blocks of P0+P1 ----------------
            p1_sb = cp.tile([4, 512], F32)
            nc.vector.tensor_copy(p1_sb[:], P1[0:4, :])
            padd = cp.tile([4, 512], F32)
            nc.vector.tensor_add(padd[:], P0[0:4, :], p1_sb[:])
            # 4 accumulating [4,128]->[128,4] transposes, each through a
            # cyclic permutation, land the diagonal sum in psum column 0
            md4 = pp.tile([128, 4], F32, tag="md")
            for n in range(4):
                nc.tensor.matmul(md4[:], padd[0:4, n * 128:(n + 1) * 128],
                                 cst32[0:4, 7 + 4 * n:11 + 4 * n],
                                 is_transpose=True,
                                 start=(n == 0), stop=(n == 3))

            # ---- LSTM tail (all [128, 1] column layout) ----------------
            mt = cp.tile([128, 1], F32)
            nc.scalar.activation(mt[:], md4[:, 0:1], AF.Tanh,
                                 scale=cst32[:, 6:7])
            t3 = cp.tile([128, 1], F32)
            nc.vector.tensor_mul(t3[:], gates[:, 3:4], mt[:])
            ct = cp.tile([128, 1], F32)
            nc.vector.tensor_add(ct[:], ct0[:], t3[:])
            tct = cp.tile([128, 1], F32)
            nc.scalar.activation(tct[:], ct[:], AF.Tanh)
            ht = cp.tile([128, 2], F32)
            nc.vector.tensor_mul(ht[:, 0:1], gates[:, 2:3], tct[:])
            nc.vector.tensor_copy(ht[:, 1:2], ct[:])

            # ---- A2C partial: hh_k = W_ih[:, slice] @ h_t_slice --------
            ht_r = cp.tile([128, 1], F32R)
            nc.vector.tensor_copy(ht_r[:], ht[:, 0:1])
            hh0 = pp.tile([1, 512], F32, tag="preA")
            hh1 = pp.tile([1, 512], F32, tag="preB")
            nc.tensor.matmul(hh0[:], ht_r[:], wiht_sb[:, 0:512])
            nc.tensor.matmul(hh1[:], ht_r[:], wiht_sb[:, 512:1024])

            # ---- outputs (out_hc = [h_t | c_t]) ------------------------
            nc.sync.dma_start(out_hc[:], ht[:])
            hh_row = cp.tile([1, H], F32)
            nc.vector.tensor_copy(hh_row[0:1, 0:512], hh0[:])
            nc.scalar.activation(hh_row[0:1, 512:1024], hh1[:], AF.Copy)
            nc.sync.dma_start(out_hh[:], hh_row[:])

    nc.compile()
    return nc


def _get_nc():
    if "nc" not in _CACHE:
        _CACHE["nc"] = _build()
    return _CACHE["nc"]


def _prep_in_maps(x_t, h, c, keys, vals, W_i2h, b_i2h, W_h2h, b_h2h,
                  W_ih, b_ih, W_actor, b_actor, W_critic, b_critic, pick_arm):
    f = np.float32
    x_t = np.asarray(x_t, f)
    h1 = np.asarray(h, f).reshape(-1)          # [H]
    c1 = np.asarray(c, f).reshape(-1)          # [H]
    keys = np.asarray(keys, f)
    vals = np.asarray(vals, f)
    W_i2h = np.asarray(W_i2h, f)
    W_h2h = np.asarray(W_h2h, f)
    W_ih = np.asarray(W_ih, f)

    # ---- host: exact softmax weights over the DND keys ------------------
    pa = int(np.asarray(pick_arm))
    start = min(max(pa * RD, 0), IN_DIM - RD)  # jax dynamic_slice clamping
    q = x_t[0, start:start + RD]
    dots = keys @ q
    kn = np.sqrt((keys * keys).sum(axis=1))
    qn = np.sqrt((q * q).sum())
    denom = np.maximum(kn * qn, np.float32(1e-8))
    s = dots / denom
    e = np.exp(s - s.max())
    w = e / e.sum()                            # [D], f32

    p2 = np.floor(np.log2(128.0 / max(float(w.max()), 1e-30)))
    wscale = np.float32(2.0 ** p2)
    winv = np.float32(1.0 / wscale)
    w_pad = np.zeros(PAD_ROWS, f)
    w_pad[:D] = w * wscale
    w_big = np.zeros((NT, 2, 32, 128), f)      # [t, ktile, stat_col, p]
    w_big[:, :, :4, :] = w_pad.reshape(NT, 2, 4, 128)
    w_t = np.ascontiguousarray(
        w_big.transpose(3, 0, 1, 2).reshape(128, NT * 64)).astype(FP8)

    # ---- vals: pad rows, fp8, chunk-tile, per-core column slices --------
    vals_f8 = np.zeros((PAD_ROWS, H), FP8)
    vals_f8[:D] = vals.astype(FP8)
    # [PAD_ROWS, H] -> [core, 128, CH * 128]
    vt = (vals_f8.reshape(CH, 128, N_CORES, 128)
          .transpose(2, 1, 0, 3).reshape(N_CORES, 128, CH * 128))

    b5 = np.asarray(b_i2h, f) + np.asarray(b_h2h, f)   # [5H]
    x_colx = np.concatenate(
        [np.zeros((IN_DIM, 640), f), x_t[0].reshape(IN_DIM, 1)], axis=1)
    h_cols = h1.reshape(8, 128).T                      # [128, 8]

    in_maps = []
    for k in range(N_CORES):
        sl = slice(128 * k, 128 * (k + 1))
        rows = (np.arange(5)[:, None] * H + np.arange(128 * k, 128 * (k + 1))
                ).ravel()                                # [640] gate rows
        w5t = (W_h2h[rows].T.reshape(8, 128, 640).transpose(1, 0, 2)
               .reshape(128, 8 * 640))
        cst16 = np.concatenate([w5t, h_cols], axis=1).astype(np.float16)
        cstx = x_colx.copy()
        cstx[:, 0:640] = W_i2h[rows].T                   # [14, 640]
        cstx = cstx.astype(np.float16)

        b5t = np.ascontiguousarray(b5[rows].reshape(5, 128).T)   # [128, 5]
        # perm n: P[m, j] = 1 iff m == (j + n) % 4  (diag sum -> column 0)
        perms = np.zeros((128, 16), f)
        for n in range(4):
            for j in range(4):
                perms[(j + n) % 4, 4 * n + j] = 1.0
        cst32 = np.concatenate(
            [b5t, c1[sl].reshape(128, 1),
             np.full((128, 1), winv, f), perms], axis=1).astype(f)

        wiht = np.ascontiguousarray(W_ih[:, sl].T).astype(f)

        in_maps.append({
            "vals_s": np.ascontiguousarray(vt[k]),
            "w_t": w_t,
            "cst16": np.ascontiguousarray(cst16),
            "cstx": np.ascontiguousarray(cstx),
            "cst32": np.ascontiguousarray(cst32),
            "wiht": wiht,
        })

    aux = {
        "b_ih": np.asarray(b_ih, f),
        "W_actor": np.asarray(W_actor, f),
        "b_actor": np.asarray(b_actor, f),
        "W_critic": np.asarray(W_critic, f),
        "b_critic": np.asarray(b_critic, f),
    }
    return in_maps, aux


def _postprocess(results, aux):
    h_t = np.concatenate([np.asarray(results[k]["out_hc"][:, 0], np.float32)
                          for k in range(N_CORES)])
    c_t = np.concatenate([np.asarray(results[k]["out_hc"][:, 1], np.float32)
                          for k in range(N_CORES)])
    hh = np.sum([np.asarray(results[k]["out_hh"][0], np.float32)
                 for k in range(N_CORES)], axis=0) + aux["b_ih"]
    hh = np.maximum(hh, 0.0)
    logits = aux["W_actor"] @ hh + aux["b_actor"]        # [A]
    v = aux["W_critic"] @ hh + aux["b_critic"]           # [1]
    m = logits.max()
    ex = np.exp(logits - m)
    pi = (ex / ex.sum()).astype(np.float32)
    a = int(np.argmax(np.log(pi) + GUMBEL))
    logp = np.float32(np.log(pi[a]))
    return np.concatenate([pi, v.astype(np.float32), [logp], h_t, c_t]
                          ).astype(np.float32)


def kernel(**inputs) -> np.ndarray:
    nc = _get_nc()
    in_maps, aux = _prep_in_maps(**inputs)
    res = run_bass_kernel_spmd(
        nc, in_maps, core_ids=list(range(N_CORES)),
        **_CACHE.get("run_kwargs", {}))
    _CACHE["last_results"] = res
    return _postprocess(res.results, aux)
